# revision 20
# baseline (speedup 1.0000x reference)
"""Trainium2 Bass kernel for FCOS-RT detection (nn_FCOS_RT_62723702391132).

Strategy:
  - The reference output depends only on image 0 of the batch (everything is
    sliced [0] before return), so only image 0 is computed.
  - Phase 1 (one SPMD NEFF on 8 cores): FPN + head convs in fp32. Sharding:
    core = (spatial quarter q = core//2) x (head type: core%2 == 0 -> cls,
    1 -> reg). Each core computes a strip-local FPN (with halo rows,
    zero-masked out-of-image) and its head stack + detection conv; per-core
    differences are pure input data, the program is uniform SPMD.
  - Phase 2: postprocess + exact greedy NMS (class-banded Jacobi sweeps).

Self-contained: all shapes hardcoded.
"""

import contextlib

import numpy as np

import concourse.bacc as bacc
import concourse.bass as bass
import concourse.tile as tile
from concourse import mybir
from concourse.masks import make_identity
from concourse.bass_utils import run_bass_kernel_spmd

F32 = mybir.dt.float32
AF = mybir.ActivationFunctionType
ALU = mybir.AluOpType

IMG = 640
NCLS = 80
CONF = 0.05
NMS_T = 0.5

_CACHE = {}


# ==========================================================================
# Phase 1 device program
# ==========================================================================

P1_INPUTS = {
    "c3s": [128, 32, 82],
    "c4s": [128, 2, 20, 42],
    "c5s": [128, 4, 15, 22],
    "lat1_t": [128, 1, 256],
    "lat2_t": [128, 2, 256],
    "lat3_t": [128, 4, 256],
    "sm1_t": [128, 9, 2, 256],
    "sm2_t": [128, 9, 2, 256],
    "sm3_t": [128, 9, 2, 256],
    "head_t": [128, 4, 9, 2, 256],
    "det_t": [128, 2, 85],
    "biases": [128, 21],
    "m3t": [6, 82], "m3b": [6, 82],
    "m4t": [5, 42], "m4b": [5, 42],
    "m5t": [5, 22], "m5b": [5, 22],
}

BIAS_COL = {"lat1": 0, "lat2": 2, "lat3": 4, "sm1": 6, "sm2": 8, "sm3": 10,
            "head0": 12, "head1": 14, "head2": 16, "head3": 18, "det": 20}


def build_phase1(dbg=()):
    nc = bacc.Bacc("TRN2", target_bir_lowering=False, debug=False,
                   enable_asserts=False, num_devices=8)
    inp = {k: nc.dram_tensor(k, shp, F32, kind="ExternalInput")
           for k, shp in P1_INPUTS.items()}
    det_out = nc.dram_tensor("det", [85, 2100], F32, kind="ExternalOutput")
    dbg_outs = {}
    DBG_SHAPES = {"p5_pre": [128, 2, 15, 22], "p5_sm": [128, 2, 15, 22],
                  "p4_pre": [128, 2, 20, 42], "p4_sm": [128, 2, 20, 42],
                  "p3_pre": [128, 2, 32, 82], "p3_sm": [128, 2, 32, 82],
                  "h3_0": [128, 2, 32, 82], "h4_0": [128, 2, 20, 42],
                  "h5_0": [128, 2, 15, 22]}
    for nm in dbg:
        dbg_outs[nm] = nc.dram_tensor("dbg_" + nm, DBG_SHAPES[nm], F32,
                                      kind="ExternalOutput")

    with tile.TileContext(nc) as tc:
        with contextlib.ExitStack() as ctx:
            _phase1_body(ctx, tc, nc, inp, det_out, dbg_outs)
    nc.compile()
    return nc


def _phase1_body(ctx, tc, nc, inp, det_out, dbg_outs=None):
    dbg_outs = dbg_outs or {}

    def dump(nm, buf):
        if nm in dbg_outs:
            nc.sync.dma_start(out=dbg_outs[nm].ap(), in_=buf)
    singles = ctx.enter_context(tc.tile_pool(name="singles", bufs=1))
    wpool = ctx.enter_context(tc.tile_pool(name="wpool", bufs=2))
    psum = ctx.enter_context(tc.tile_pool(name="psum", bufs=6, space="PSUM"))

    def load(name):
        t = singles.tile(P1_INPUTS[name], F32, tag=f"in_{name}")
        nc.sync.dma_start(out=t, in_=inp[name].ap())
        return t

    c3s, c4s, c5s = load("c3s"), load("c4s"), load("c5s")
    lat1_t, lat2_t, lat3_t = load("lat1_t"), load("lat2_t"), load("lat3_t")
    det_t, biases = load("det_t"), load("biases")

    masks = {}
    for nm in ("m3t", "m3b", "m4t", "m4b", "m5t", "m5b"):
        rows, w = P1_INPUTS[nm]
        m = singles.tile([128, rows, w], F32, tag=f"mask_{nm}")
        src = inp[nm].ap()
        bcast = bass.AP(tensor=src.tensor, offset=src.offset,
                        ap=[[0, 128]] + [list(p) for p in src.ap])
        nc.sync.dma_start(out=m, in_=bcast)
        masks[nm] = m

    # working buffers [128, 2(cout), R, W+2]; zeroed once (pad cols stay 0)
    bufs = {}
    for nm, shape in (("pA", [128, 2, 32, 82]), ("pB", [128, 2, 32, 82]),
                      ("qA", [128, 2, 20, 42]), ("qB", [128, 2, 20, 42]),
                      ("rA", [128, 2, 15, 22]), ("rB", [128, 2, 15, 22])):
        b = singles.tile(shape, F32, tag=f"buf_{nm}")
        nc.vector.memset(b, 0.0)
        bufs[nm] = b
    det_sb = singles.tile([85, 2100], F32, tag="det_sb")

    def bias_ap(nm, co):
        c = BIAS_COL[nm] + co
        return biases[:, c:c + 1]

    def conv3(out_buf, out_rows, in_buf, in_row0, w_tile, nkc, W, bias_name,
              leaky):
        """3x3 valid conv over rows: out rows [0, out_rows) from in rows
        [in_row0, in_row0 + out_rows + 2). Buffers [128, 2, R, W+2]."""
        maxr = max(1, 480 // W)
        for co in range(2):
            r = 0
            while r < out_rows:
                rows = min(maxr, out_rows - r)
                ps = psum.tile([128, maxr, W], F32, tag="convps")
                k = 0
                for tap in range(9):
                    dy, dx = tap // 3, tap % 3
                    for kc in range(nkc):
                        rhs = in_buf[:, kc, in_row0 + r + dy:
                                     in_row0 + r + dy + rows, dx:dx + W]
                        nc.tensor.matmul(
                            ps[:, :rows, :],
                            w_tile[:, tap, kc, co * 128:(co + 1) * 128],
                            rhs, start=(k == 0), stop=(k == 9 * nkc - 1))
                        k += 1
                nc.scalar.activation(
                    out_buf[:, co, r:r + rows, 1:1 + W], ps[:, :rows, :],
                    AF.Prelu if leaky else AF.Identity,
                    bias=bias_ap(bias_name, co), alpha=0.1 if leaky else 0.0)
                r += rows

    def lat_conv(out_buf, out_R, in_buf, lat_t, nkc, W, bias_name):
        maxr = max(1, 480 // W)
        for co in range(2):
            r = 0
            while r < out_R:
                rows = min(maxr, out_R - r)
                ps = psum.tile([128, maxr, W], F32, tag="convps")
                for kc in range(nkc):
                    rhs = in_buf[:, kc, r:r + rows, 1:1 + W]
                    nc.tensor.matmul(ps[:, :rows, :],
                                     lat_t[:, kc, co * 128:(co + 1) * 128],
                                     rhs, start=(kc == 0), stop=(kc == nkc - 1))
                nc.scalar.activation(out_buf[:, co, r:r + rows, 1:1 + W],
                                     ps[:, :rows, :], AF.Identity,
                                     bias=bias_ap(bias_name, co))
                r += rows

    def apply_masks(buf, level, R, delta):
        h = 6 if level == 3 else 5
        k = h - delta
        if k <= 0:
            return
        w = buf.shape[3]
        mt, mb = masks[f"m{level}t"], masks[f"m{level}b"]
        top = mt[:, delta:h, :].unsqueeze(1).broadcast_to([128, 2, k, w])
        nc.vector.tensor_tensor(out=buf[:, :, 0:k, :], in0=buf[:, :, 0:k, :],
                                in1=top, op=ALU.mult)
        bot = mb[:, 0:k, :].unsqueeze(1).broadcast_to([128, 2, k, w])
        nc.vector.tensor_tensor(out=buf[:, :, R - k:R, :],
                                in0=buf[:, :, R - k:R, :], in1=bot, op=ALU.mult)

    pA, pB = bufs["pA"], bufs["pB"]
    qA, qB = bufs["qA"], bufs["qB"]
    rA, rB = bufs["rA"], bufs["rB"]

    # ---------------- FPN ----------------
    # p5_pre = lat3(c5s) : rA rows [0,15) ~ global [5q-5, 5q+10)
    lat_conv(rA, 15, c5s, lat3_t, 4, 20, "lat3")
    apply_masks(rA, 5, 15, 0)
    dump("p5_pre", rA)

    # p5_sm = sm3(p5_pre): rB rows [0,13) ~ global [5q-4, 5q+9)
    sm3_w = singles.tile([128, 9, 2, 256], F32, tag="sm3w")
    nc.sync.dma_start(out=sm3_w, in_=inp["sm3_t"].ap())
    conv3(rB, 13, rA, 0, sm3_w, 2, 20, "sm3", False)
    apply_masks(rB, 5, 13, 1)
    dump("p5_sm", rB)

    # lat2(c4) + bias -> qA rows [0,20) ~ global [10q-5, 10q+15)
    lat_conv(qA, 20, c4s, lat2_t, 2, 40, "lat2")
    # qA += up2(p5_pre): p4 row r4 <- p5 local row (r4+5)//2
    for co in range(2):
        # even rows r4 = 0,2,..,18 <- p5 rows 2..11
        dst_e = qA[:, co, 0:20:2, 1:41].rearrange(
            "p r (w two) -> p r w two", two=2)
        src_e = rA[:, co, 2:12, 1:21].unsqueeze(3).broadcast_to([128, 10, 20, 2])
        nc.vector.tensor_tensor(out=dst_e, in0=dst_e, in1=src_e, op=ALU.add)
        # odd rows r4 = 1,3,..,19 <- p5 rows 3..12
        dst_o = qA[:, co, 1:20:2, 1:41].rearrange(
            "p r (w two) -> p r w two", two=2)
        src_o = rA[:, co, 3:13, 1:21].unsqueeze(3).broadcast_to([128, 10, 20, 2])
        nc.vector.tensor_tensor(out=dst_o, in0=dst_o, in1=src_o, op=ALU.add)
    apply_masks(qA, 4, 20, 0)
    dump("p4_pre", qA)

    # p4_sm = sm2(p4_pre): qB rows [0,18) ~ global [10q-4, 10q+14)
    sm2_w = wpool.tile([128, 9, 2, 256], F32, tag="w")
    nc.sync.dma_start(out=sm2_w, in_=inp["sm2_t"].ap())
    conv3(qB, 18, qA, 0, sm2_w, 2, 40, "sm2", False)
    apply_masks(qB, 4, 18, 1)
    dump("p4_sm", qB)

    # lat1(c3) + bias -> pA rows [0,32) ~ global [20q-6, 20q+26)
    c3v = c3s[:, :, :].unsqueeze(1)  # [128, 1, 32, 82]
    lat_conv(pA, 32, c3v, lat1_t, 1, 80, "lat1")
    # pA += up2(p4_pre rows [2,18)): pair aligned; split by row parity to
    # keep APs at <=3 free dims
    for co in range(2):
        src = qA[:, co, 2:18, 1:41].unsqueeze(3).broadcast_to([128, 16, 40, 2])
        for par in range(2):
            dst = pA[:, co, par:32:2, 1:81].rearrange(
                "p r (w two) -> p r w two", two=2)
            nc.vector.tensor_tensor(out=dst, in0=dst, in1=src, op=ALU.add)
    apply_masks(pA, 3, 32, 0)
    dump("p3_pre", pA)

    # p3_sm = sm1(p3_pre): pB rows [0,30) ~ global [20q-5, 20q+25)
    sm1_w = wpool.tile([128, 9, 2, 256], F32, tag="w")
    nc.sync.dma_start(out=sm1_w, in_=inp["sm1_t"].ap())
    conv3(pB, 30, pA, 0, sm1_w, 2, 80, "sm1", False)
    apply_masks(pB, 3, 30, 1)
    dump("p3_sm", pB)

    # ---------------- heads ----------------
    # level state: (cur buffer, next buffer, rows, in_row0 for conv1)
    st = {3: [pB, pA, 28, 1], 4: [qB, qA, 18, 0], 5: [rB, rA, 13, 0]}
    for ci in range(4):
        hw = wpool.tile([128, 9, 2, 256], F32, tag="w")
        nc.sync.dma_start(out=hw, in_=inp["head_t"].ap()[:, ci, :, :, :])
        for lv in (5, 4, 3):
            cur, nxt, rows, r0 = st[lv]
            W = {3: 80, 4: 40, 5: 20}[lv]
            out_rows = rows - 2
            conv3(nxt, out_rows, cur, r0, hw, 2, W, f"head{ci}", True)
            delta = {3: 3, 4: 2, 5: 2}[lv] + ci
            if ci < 3:
                apply_masks(nxt, lv, out_rows, delta)
            if ci == 0:
                dump(f"h{lv}_0", nxt)
            st[lv] = [nxt, cur, out_rows, 0]

    # ---------------- det conv (1x1, 85 outputs) ----------------
    det_base = {3: 0, 4: 1600, 5: 2000}
    for lv in (3, 4, 5):
        cur, _, rows, _ = st[lv]
        W = {3: 80, 4: 40, 5: 20}[lv]
        base = det_base[lv]
        maxr = max(1, 480 // W)
        r = 0
        while r < rows:
            rr = min(maxr, rows - r)
            ps = psum.tile([128, maxr, W], F32, tag="convps")
            for kc in range(2):
                rhs = cur[:, kc, r:r + rr, 1:1 + W]
                nc.tensor.matmul(ps[:85, :rr, :], det_t[:, kc, :], rhs,
                                 start=(kc == 0), stop=(kc == 1))
            out_view = det_sb[:, base + r * W: base + (r + rr) * W].rearrange(
                "p (r w) -> p r w", w=W)
            nc.scalar.activation(out_view, ps[:85, :rr, :], AF.Identity,
                                 bias=bias_ap("det", 0)[:85, :])
            r += rr

    nc.sync.dma_start(out=det_out.ap(), in_=det_sb)


# ==========================================================================
# Host-side data prep for phase 1
# ==========================================================================

def _slice_rows(x, lo, hi):
    C, H, W = x.shape
    out = np.zeros((C, hi - lo, W), np.float32)
    a, b = max(lo, 0), min(hi, H)
    if a < b:
        out[:, a - lo:b - lo] = x[:, a:b]
    return out


def _wpad(x):
    # [C?, R, W] -> [..., R, W+2] zero cols
    pad = [(0, 0)] * (x.ndim - 1) + [(1, 1)]
    return np.pad(x, pad)


def _prep_weights(inputs):
    """Common (head-independent) weight prep -> dict of arrays."""
    out = {}
    lat1 = np.asarray(inputs["lat1_w"])[:, :, 0, 0]     # [256, 128]
    lat2 = np.asarray(inputs["lat2_w"])[:, :, 0, 0]
    lat3 = np.asarray(inputs["lat3_w"])[:, :, 0, 0]
    out["lat1_t"] = np.ascontiguousarray(lat1.T)[:, None, :]            # [128,1,256]
    out["lat2_t"] = np.ascontiguousarray(
        lat2.T.reshape(2, 128, 256).transpose(1, 0, 2))                 # [128,2,256]
    out["lat3_t"] = np.ascontiguousarray(
        lat3.T.reshape(4, 128, 256).transpose(1, 0, 2))                 # [128,4,256]
    for nm in ("sm1", "sm2", "sm3"):
        w = np.asarray(inputs[nm + "_w"])                               # [256,256,3,3]
        t = np.zeros((128, 9, 2, 256), np.float32)
        for tap in range(9):
            dy, dx = tap // 3, tap % 3
            wt = w[:, :, dy, dx].T.reshape(2, 128, 256)                 # [kc,k,cout]
            t[:, tap] = wt.transpose(1, 0, 2)
        out[nm + "_t"] = t
    return out


def _prep_head(head_w):
    """head_w [4, 256, 256, 3, 3] -> [128, 4, 9, 2, 256]."""
    t = np.zeros((128, 4, 9, 2, 256), np.float32)
    for ci in range(4):
        for tap in range(9):
            dy, dx = tap // 3, tap % 3
            wt = head_w[ci, :, :, dy, dx].T.reshape(2, 128, 256)
            t[:, ci, tap] = wt.transpose(1, 0, 2)
    return t


def _prep_det(det_w85):
    """det_w85 [85, 256] -> [128, 2, 85]."""
    t = det_w85.T.reshape(2, 128, 85)                                   # [kc,k,m]
    return np.ascontiguousarray(t.transpose(1, 0, 2))


def _prep_biases(inputs, head, det_b85):
    b = np.zeros((128, 21), np.float32)

    def put(col, vec):
        v = np.asarray(vec, np.float32)
        n = min(128, len(v))
        b[:n, col] = v[:n]
        if len(v) > 128:
            b[:, col + 1] = v[128:256]

    put(BIAS_COL["lat1"], inputs["lat1_b"])
    put(BIAS_COL["lat2"], inputs["lat2_b"])
    put(BIAS_COL["lat3"], inputs["lat3_b"])
    put(BIAS_COL["sm1"], inputs["sm1_b"])
    put(BIAS_COL["sm2"], inputs["sm2_b"])
    put(BIAS_COL["sm3"], inputs["sm3_b"])
    hb = np.asarray(inputs["clsh_b" if head == "cls" else "regh_b"])    # [4,256]
    for ci in range(4):
        put(BIAS_COL[f"head{ci}"], hb[ci])
    b[:85, BIAS_COL["det"]] = det_b85
    return b


def _prep_masks(q):
    out = {}
    for lv, (step, H, W, h) in {3: (20, 80, 80, 6), 4: (10, 40, 40, 5),
                                5: (5, 20, 20, 5)}.items():
        lo0 = step * q - h
        mt = np.ones((h, W + 2), np.float32)
        for i in range(h):
            if lo0 + i < 0:
                mt[i] = 0
        # bottom anchor: global = step*q + (step*4? ...) per derivation:
        # bottom window start global = step*q + (hi0 - h) - ... = step*q + H/4*?
        # hi0 = step*q + (crows - h_top)??  derived: start = step*q + step + (h - ...)
        bot0 = {3: 20 * q + 20, 4: 10 * q + 10, 5: 5 * q + 5}[lv]
        mb = np.ones((h, W + 2), np.float32)
        for i in range(h):
            if bot0 + i >= H:
                mb[i] = 0
        out[f"m{lv}t"] = mt
        out[f"m{lv}b"] = mb
    return out


def prep_phase1_inputs(inputs):
    """Returns list of 8 in_maps."""
    c3 = np.asarray(inputs["c3"])[0]
    c4 = np.asarray(inputs["c4"])[0]
    c5 = np.asarray(inputs["c5"])[0]
    common = _prep_weights(inputs)

    cls_det = np.asarray(inputs["cls_det_w"])[:, :, 0, 0]
    reg_det = np.asarray(inputs["reg_det_w"])[:, :, 0, 0]
    ctn_det = np.asarray(inputs["ctn_det_w"])[:, :, 0, 0]
    det85 = {
        "cls": np.pad(cls_det, ((0, 5), (0, 0))).astype(np.float32),
        "reg": np.pad(np.concatenate([reg_det, ctn_det], 0),
                      ((0, 80), (0, 0))).astype(np.float32),
    }
    detb85 = {
        "cls": np.pad(np.asarray(inputs["cls_det_b"]), (0, 5)).astype(np.float32),
        "reg": np.pad(np.concatenate([np.asarray(inputs["reg_det_b"]),
                                      np.asarray(inputs["ctn_det_b"])]),
                      (0, 80)).astype(np.float32),
    }
    head_t = {
        "cls": _prep_head(np.asarray(inputs["clsh_w"])),
        "reg": _prep_head(np.asarray(inputs["regh_w"])),
    }

    in_maps = []
    for core in range(8):
        q, head = core // 2, ("cls", "reg")[core % 2]
        m = dict(common)
        c3q = _wpad(_slice_rows(c3, 20 * q - 6, 20 * q + 26))           # [128,32,82]
        m["c3s"] = np.ascontiguousarray(c3q)
        c4q = _wpad(_slice_rows(c4, 10 * q - 5, 10 * q + 15))           # [256,20,42]
        m["c4s"] = np.ascontiguousarray(
            c4q.reshape(2, 128, 20, 42).transpose(1, 0, 2, 3))
        c5q = _wpad(_slice_rows(c5, 5 * q - 5, 5 * q + 10))             # [512,15,22]
        m["c5s"] = np.ascontiguousarray(
            c5q.reshape(4, 128, 15, 22).transpose(1, 0, 2, 3))
        m["head_t"] = head_t[head]
        m["det_t"] = _prep_det(det85[head])
        m["biases"] = _prep_biases(inputs, head, detb85[head])
        m.update(_prep_masks(q))
        in_maps.append({k: np.ascontiguousarray(v, dtype=np.float32)
                        for k, v in m.items()})
    return in_maps


def assemble_phase1(results):
    """results: list of 8 dicts with 'det' [85, 2100] ->
    cls [8400, 80], reg [8400, 4], ctn [8400]."""
    def asm(dets, nch):
        lv3 = np.concatenate(
            [d[:nch, :1600].reshape(nch, 20, 80) for d in dets], axis=1)
        lv4 = np.concatenate(
            [d[:nch, 1600:2000].reshape(nch, 10, 40) for d in dets], axis=1)
        lv5 = np.concatenate(
            [d[:nch, 2000:].reshape(nch, 5, 20) for d in dets], axis=1)
        return np.concatenate([lv3.reshape(nch, -1), lv4.reshape(nch, -1),
                               lv5.reshape(nch, -1)], axis=1)

    cls = asm([results[2 * q]["det"] for q in range(4)], 80).T
    regctn = asm([results[2 * q + 1]["det"] for q in range(4)], 5)
    return (np.ascontiguousarray(cls),
            np.ascontiguousarray(regctn[:4].T), regctn[4].copy())


# ==========================================================================
# Phase 2 (temporary numpy implementation; device port in progress)
# ==========================================================================

def _grids():
    g, s = [], []
    for st in (8, 16, 32):
        hs = IMG // st
        gy, gx = np.meshgrid(np.arange(hs), np.arange(hs), indexing="ij")
        g.append(np.stack([gx, gy], -1).reshape(-1, 2).astype(np.float32))
        s.append(np.full((hs * hs,), st, np.float32))
    return np.concatenate(g, 0), np.concatenate(s)


def _sigmoid(x):
    return 1.0 / (1.0 + np.exp(-x))


def phase2_numpy(cls, reg, ctn, n_sweeps=4, band=1024):
    N = cls.shape[0]
    g, s = _grids()
    x1y1 = (g - np.exp(reg[:, :2])) * s[:, None]
    x2y2 = (g + np.exp(reg[:, 2:])) * s[:, None]
    bboxes = np.clip(np.concatenate([x1y1, x2y2], 1) / IMG, 0.0, 1.0) \
        .astype(np.float32)
    cls_inds = np.argmax(cls, axis=1).astype(np.int32)
    best_logit = cls[np.arange(N), cls_inds]
    p = (_sigmoid(best_logit) * _sigmoid(ctn)).astype(np.float32)
    best = np.sqrt(p).astype(np.float32)
    conf = p >= np.float32(CONF) ** 2

    c = cls_inds.astype(np.float32)
    x1, y1, x2, y2 = bboxes.T
    fx1 = x1 + 2 * c
    fx2 = x2 + 2 * c
    a3 = ((x2 - x1) * (y2 - y1) * (NMS_T / (1 + NMS_T))).astype(np.float32)

    order = np.argsort(cls_inds, kind="stable")
    Npad = 8448
    sx1 = np.full(Npad, 1e6, np.float32); sx2 = np.full(Npad, 1e6 + 1, np.float32)
    sy1 = np.zeros(Npad, np.float32);     sy2 = np.ones(Npad, np.float32)
    sa3 = np.full(Npad, 1e9, np.float32); sp = np.full(Npad, 2.0, np.float32)
    sconf = np.zeros(Npad, bool)
    sx1[:N] = fx1[order]; sx2[:N] = fx2[order]
    sy1[:N] = y1[order];  sy2[:N] = y2[order]
    sa3[:N] = a3[order];  sp[:N] = p[order]
    sconf[:N] = conf[order]

    kept = sconf.copy()
    for _ in range(n_sweeps):
        tk = np.where(kept, sp, 0.0).astype(np.float32)
        sup = np.zeros(Npad, bool)
        for t0 in range(0, Npad, 128):
            rows = slice(t0, t0 + 128)
            c0 = max(0, t0 - band); c1 = min(Npad, t0 + 128 + band)
            w = np.minimum(sx2[c0:c1][None, :], sx2[rows][:, None]) + \
                np.minimum(-sx1[c0:c1][None, :], -sx1[rows][:, None])
            h = np.minimum(sy2[c0:c1][None, :], sy2[rows][:, None]) + \
                np.minimum(-sy1[c0:c1][None, :], -sy1[rows][:, None])
            inter = np.maximum(w, 0) * np.maximum(h, 0)
            t = (inter - sa3[rows][:, None]) > sa3[c0:c1][None, :]
            contrib = (tk[c0:c1][None, :] > sp[rows][:, None]) & t
            sup[rows] = contrib.sum(axis=1) > 0
        kept = sconf & ~sup
    keep = np.zeros(N, bool)
    keep[order] = kept[:N]
    return bboxes, best, cls_inds, keep



# ==========================================================================
# Phase 2 device program: postprocess + all-pairs NMS (S matrix + PE sweeps)
# ==========================================================================

T72 = 72          # padded plane columns (anchors a=(p,t): a = p*66+t, t<66)
NSWEEP = 3

P2_INPUTS = {
    "cls_t": ([128, T72, 80], F32),
    "ctn_t": ([128, T72], F32),
    "reg_t": ([128, T72, 4], F32),
    "gx": ([128, T72], F32),
    "gy": ([128, T72], F32),
    "sv": ([128, T72], F32),
    "iota_mb": ([128, 80], F32),
    "ibase": ([1, 1], mybir.dt.uint32),
}


def build_phase2():
    nc = bacc.Bacc("TRN2", target_bir_lowering=False, debug=False,
                   enable_asserts=False, num_devices=8)
    inp = {k: nc.dram_tensor(k, shp, dt, kind="ExternalInput")
           for k, (shp, dt) in P2_INPUTS.items()}
    outs = {
        "box_pl": nc.dram_tensor("box_pl", [128, T72, 4], F32, kind="ExternalOutput"),
        "best_pl": nc.dram_tensor("best_pl", [128, T72], F32, kind="ExternalOutput"),
        "cls_pl": nc.dram_tensor("cls_pl", [128, T72], mybir.dt.int32, kind="ExternalOutput"),
        "keep_pl": nc.dram_tensor("keep_pl", [128, T72], F32, kind="ExternalOutput"),
    }
    with tile.TileContext(nc) as tc:
        with contextlib.ExitStack() as ctx:
            _phase2_body(ctx, tc, nc, inp, outs)
    nc.compile()
    return nc


def _phase2_body(ctx, tc, nc, inp, outs):
    FP8 = mybir.dt.float8e4
    sp = ctx.enter_context(tc.tile_pool(name="sp", bufs=1))
    tp = ctx.enter_context(tc.tile_pool(name="tp", bufs=3))
    ps_pool = ctx.enter_context(tc.tile_pool(name="ps", bufs=2, space="PSUM"))
    dram = ctx.enter_context(tc.tile_pool(name="dram", bufs=2, space="DRAM"))

    def load(name):
        shp, dt = P2_INPUTS[name]
        t = sp.tile(shp, dt, tag=f"in_{name}")
        nc.sync.dma_start(out=t, in_=inp[name].ap())
        return t

    cls_t, ctn_t, reg_t = load("cls_t"), load("ctn_t"), load("reg_t")
    gx, gy, sv = load("gx"), load("gy"), load("sv")
    iota_mb, ibase_t = load("iota_mb"), load("ibase")

    regs = nc.alloc_registers("ibase")
    nc.regs_load(regs, ibase_t[0:1, 0:1])
    ib = nc.snap(regs, donate=True, min_val=0, max_val=63)

    def plane(tag):
        return sp.tile([128, T72], F32, tag=tag, name=tag)

    # ---- argmax over classes ----
    rmax = plane("rmax")
    nc.vector.tensor_reduce(rmax, cls_t[:, :, :], axis=mybir.AxisListType.X,
                            op=ALU.max)
    e = sp.tile([128, T72, 80], F32, tag="e")
    nc.vector.tensor_tensor(out=e, in0=cls_t[:, :, :],
                            in1=rmax[:, :].unsqueeze(2).broadcast_to([128, T72, 80]),
                            op=ALU.is_ge)
    nc.vector.tensor_tensor(out=e, in0=e,
                            in1=iota_mb[:, :].unsqueeze(1).broadcast_to([128, T72, 80]),
                            op=ALU.mult)
    idxm = plane("idxm")
    nc.vector.tensor_reduce(idxm, e, axis=mybir.AxisListType.X, op=ALU.min)
    cidx = plane("cidx")
    nc.vector.tensor_scalar(out=cidx, in0=idxm, scalar1=1.0e6, scalar2=None,
                            op0=ALU.add)
    cls_pl_sb = sp.tile([128, T72], mybir.dt.int32, tag="cls_pl_sb")
    nc.vector.tensor_copy(cls_pl_sb, cidx)
    nc.sync.dma_start(out=outs["cls_pl"].ap(), in_=cls_pl_sb)

    # ---- priority, conf, best ----
    sigc, sigt, p = plane("sigc"), plane("sigt"), plane("p")
    nc.scalar.activation(sigc, rmax, AF.Sigmoid)
    nc.scalar.activation(sigt, ctn_t, AF.Sigmoid)
    nc.vector.tensor_tensor(out=p, in0=sigc, in1=sigt, op=ALU.mult)
    conf = plane("conf")
    nc.vector.tensor_scalar(out=conf, in0=p, scalar1=float(CONF) ** 2,
                            scalar2=None, op0=ALU.is_ge)
    r0, rec, best = plane("r0"), plane("rec"), plane("best")
    nc.scalar.activation(r0, p, AF.Sqrt)
    nc.vector.tensor_scalar(out=rec, in0=r0, scalar1=1e-20, scalar2=None,
                            op0=ALU.max)
    nc.vector.reciprocal(rec, rec)
    nc.vector.scalar_tensor_tensor(out=rec, in0=p, scalar=1.0, in1=rec,
                                   op0=ALU.mult, op1=ALU.mult)
    nc.vector.tensor_tensor(out=best, in0=rec, in1=r0, op=ALU.add)
    nc.vector.tensor_scalar(out=best, in0=best, scalar1=0.5, scalar2=None,
                            op0=ALU.mult)
    nc.sync.dma_start(out=outs["best_pl"].ap(), in_=best)

    # ---- box decode ----
    ex = sp.tile([128, T72, 4], F32, tag="ex")
    nc.scalar.activation(ex, reg_t[:, :, :], AF.Exp)
    coords = {}
    for nm, gi, ei, sign in (("x1", gx, 0, -1.0), ("y1", gy, 1, -1.0),
                             ("x2", gx, 2, 1.0), ("y2", gy, 3, 1.0)):
        c = plane("c_" + nm)
        nc.vector.scalar_tensor_tensor(out=c, in0=ex[:, :, ei], scalar=sign,
                                       in1=gi, op0=ALU.mult, op1=ALU.add)
        nc.vector.tensor_tensor(out=c, in0=c, in1=sv, op=ALU.mult)
        nc.vector.tensor_scalar(out=c, in0=c, scalar1=1.0 / IMG, scalar2=1.0,
                                op0=ALU.mult, op1=ALU.min)
        nc.vector.tensor_scalar(out=c, in0=c, scalar1=0.0, scalar2=None,
                                op0=ALU.max)
        nc.sync.dma_start(out=outs["box_pl"].ap()[:, :, ei], in_=c)
        coords[nm] = c
    x1c, y1c, x2c, y2c = coords["x1"], coords["y1"], coords["x2"], coords["y2"]

    # ---- folded coords + a3 ----
    x1f, x2f = plane("x1f"), plane("x2f")
    nc.vector.scalar_tensor_tensor(out=x1f, in0=cidx, scalar=2.0, in1=x1c,
                                   op0=ALU.mult, op1=ALU.add)
    nc.vector.scalar_tensor_tensor(out=x2f, in0=cidx, scalar=2.0, in1=x2c,
                                   op0=ALU.mult, op1=ALU.add)
    nx1f, ny1 = plane("nx1f"), plane("ny1")
    nc.vector.tensor_scalar(out=nx1f, in0=x1f, scalar1=-1.0, scalar2=None,
                            op0=ALU.mult)
    nc.vector.tensor_scalar(out=ny1, in0=y1c, scalar1=-1.0, scalar2=None,
                            op0=ALU.mult)
    wd, ht, a3 = plane("wd"), plane("ht"), plane("a3")
    nc.vector.tensor_tensor(out=wd, in0=x2c, in1=x1c, op=ALU.subtract)
    nc.vector.tensor_tensor(out=ht, in0=y2c, in1=y1c, op=ALU.subtract)
    nc.vector.scalar_tensor_tensor(out=a3, in0=wd, scalar=1.0 / 3.0, in1=ht,
                                   op0=ALU.mult, op1=ALU.mult)

    ones1 = sp.tile([1, 128], F32, tag="ones1")
    nc.vector.memset(ones1, 1.0)
    ident = sp.tile([128, 128], F32, tag="ident")
    make_identity(nc, ident)

    # ---- sweep 0: materialize S [j, i] fp8, band = all pairs ----
    # i-shard: 9 plane columns starting at ibase (=9*core), 3 supertiles x 3
    S = sp.tile([128, 3, T72, 384], FP8, tag="S")
    iarr = (x2f, nx1f, y2c, ny1, a3, p)
    for sti in range(3):
        reps = []
        for ai, arr in enumerate(iarr):
            loc = tp.tile([128, 3], F32, tag="loc", name="loc")
            nc.vector.tensor_copy(loc, arr[:, bass.ds(ib + 3 * sti, 3)])
            rep_ps = ps_pool.tile([128, 384], F32, tag="repps")
            for cc in range(3):
                tr_ps = ps_pool.tile([128, 128], F32, tag="trps")
                nc.tensor.transpose(tr_ps[0:1, :], loc[:, cc:cc + 1], ident)
                tr1 = tp.tile([1, 128], F32, tag="tr1", name="tr1")
                nc.vector.tensor_copy(tr1, tr_ps[0:1, :])
                nc.tensor.matmul(rep_ps[:, cc * 128:(cc + 1) * 128], ones1,
                                 tr1, start=True, stop=True)
            rep_s = tp.tile([128, 384], F32, tag=f"rep_{ai}")
            nc.vector.tensor_copy(rep_s, rep_ps)
            reps.append(rep_s)
        rx2f, rnx1f, ry2, rny1, ra3, rp = reps
        for jc in range(T72):
            m1 = tp.tile([128, 384], F32, tag="m1")
            nc.vector.tensor_scalar(out=m1, in0=rx2f, scalar1=x2f[:, jc:jc + 1],
                                    scalar2=None, op0=ALU.min)
            w = tp.tile([128, 384], F32, tag="w")
            nc.vector.scalar_tensor_tensor(out=w, in0=rnx1f,
                                           scalar=nx1f[:, jc:jc + 1], in1=m1,
                                           op0=ALU.min, op1=ALU.add)
            m3 = tp.tile([128, 384], F32, tag="m3")
            nc.vector.tensor_scalar(out=m3, in0=ry2, scalar1=y2c[:, jc:jc + 1],
                                    scalar2=None, op0=ALU.min)
            h = tp.tile([128, 384], F32, tag="h")
            nc.vector.scalar_tensor_tensor(out=h, in0=rny1,
                                           scalar=ny1[:, jc:jc + 1], in1=m3,
                                           op0=ALU.min, op1=ALU.add)
            hr = tp.tile([128, 384], F32, tag="hr")
            nc.scalar.activation(hr, h, AF.Relu)
            inter = tp.tile([128, 384], F32, tag="inter")
            nc.vector.scalar_tensor_tensor(out=inter, in0=w, scalar=0.0, in1=hr,
                                           op0=ALU.max, op1=ALU.mult)
            tgeo = tp.tile([128, 384], F32, tag="tgeo")
            nc.vector.scalar_tensor_tensor(out=tgeo, in0=inter,
                                           scalar=a3[:, jc:jc + 1], in1=ra3,
                                           op0=ALU.subtract, op1=ALU.is_gt)
            nc.vector.scalar_tensor_tensor(out=S[:, sti, jc, :], in0=rp,
                                           scalar=p[:, jc:jc + 1], in1=tgeo,
                                           op0=ALU.is_lt, op1=ALU.mult)

    # ---- Jacobi sweeps via PE matvec ----
    kept = plane("kept")
    nc.vector.tensor_copy(kept, conf)
    shard_d = dram.tile([9, 128], F32, tag="shard")
    full_d = dram.tile([T72, 128], F32, tag="full")
    for sw in range(NSWEEP):
        k8 = sp.tile([128, T72], FP8, tag="k8")
        nc.vector.tensor_copy(k8, kept)
        kl = sp.tile([128, 9], F32, tag="kl")
        for it in range(9):
            mv = ps_pool.tile([128, 1], F32, tag="mv")
            sti, cc = it // 3, it % 3
            for jc in range(T72):
                nc.tensor.matmul(mv, S[:, sti, jc, cc * 128:(cc + 1) * 128],
                                 k8[:, jc:jc + 1], start=(jc == 0),
                                 stop=(jc == T72 - 1))
            k1 = tp.tile([128, 1], F32, tag="k1")
            nc.vector.tensor_scalar(out=k1, in0=mv, scalar1=0.5, scalar2=None,
                                    op0=ALU.is_lt)
            nc.vector.tensor_tensor(out=kl[:, it:it + 1], in0=k1,
                                    in1=conf[:, bass.ds(ib + it, 1)],
                                    op=ALU.mult)
        # transpose kept_local -> [9, 128], AllGather, reload full
        tr_ps = ps_pool.tile([128, 128], F32, tag="trps")
        nc.tensor.transpose(tr_ps[0:9, :], kl, ident)
        klT = tp.tile([9, 128], F32, tag="klT")
        nc.vector.tensor_copy(klT, tr_ps[0:9, :])
        nc.sync.dma_start(out=shard_d, in_=klT)
        nc.gpsimd.collective_compute(
            "AllGather", ALU.bypass, replica_groups=[list(range(8))],
            ins=[shard_d.opt()], outs=[full_d.opt()])
        fl = sp.tile([T72, 128], F32, tag="fl")
        nc.sync.dma_start(out=fl, in_=full_d)
        tr2 = ps_pool.tile([128, 128], F32, tag="tr2ps")
        nc.tensor.transpose(tr2[:, 0:T72], fl, ident[0:T72, 0:T72])
        nc.vector.tensor_copy(kept, tr2[:, 0:T72])
    nc.sync.dma_start(out=outs["keep_pl"].ap(), in_=kept)


def prep_phase2_inputs(cls, reg, ctn):
    """cls [8400, 80], reg [8400, 4], ctn [8400] -> 8 in_maps."""
    N, Npl = 8400, 128 * 66
    def to_plane(x, pad_val):
        # x [8400, ...] -> [128, 72, ...] with a = p*66 + t, pads at t>=66
        tail = x.shape[1:]
        xp = np.full((Npl,) + tail, pad_val, np.float32)
        xp[:N] = x
        full = np.full((128, T72) + tail, pad_val, np.float32)
        full[:, :66] = xp.reshape(128, 66, *tail)
        return full

    g, s = _grids()
    in0 = {
        "cls_t": to_plane(cls, -100.0),
        "ctn_t": to_plane(ctn[:, None], -100.0)[:, :, 0],
        "reg_t": to_plane(reg, 0.0),
        "gx": to_plane(g[:, 0:1], 1e5)[:, :, 0],
        "gy": to_plane(g[:, 1:2], 1e5)[:, :, 0],
        "sv": to_plane(s[:, None], 1.0)[:, :, 0],
        "iota_mb": np.tile(np.arange(80).astype(np.float32) - 1.0e6, (128, 1)),
    }
    maps = []
    for c in range(8):
        m = dict(in0)
        m["ibase"] = np.array([[9 * c]], np.uint32)
        maps.append({k: np.ascontiguousarray(v) for k, v in m.items()})
    return maps


def unpack_phase2(res):
    box = res["box_pl"][:, :66].reshape(8448, 4)[:8400]
    best = res["best_pl"][:, :66].reshape(8448)[:8400]
    cls_i = res["cls_pl"][:, :66].reshape(8448)[:8400].astype(np.int32)
    keep = res["keep_pl"][:, :66].reshape(8448)[:8400] > 0.5
    return box, best, cls_i, keep


# ==========================================================================
# Entry point
# ==========================================================================

def kernel(**inputs):
    if "p1" not in _CACHE:
        _CACHE["p1"] = build_phase1()
    if "p2" not in _CACHE:
        _CACHE["p2"] = build_phase2()
    in_maps = prep_phase1_inputs(inputs)
    r1 = run_bass_kernel_spmd(_CACHE["p1"], in_maps, core_ids=list(range(8)))
    cls, reg, ctn = assemble_phase1(r1.results)
    maps2 = prep_phase2_inputs(cls, reg, ctn)
    r2 = run_bass_kernel_spmd(_CACHE["p2"], maps2, core_ids=list(range(8)))
    bboxes, best, cls_inds, keep = unpack_phase2(r2.results[0])
    return bboxes, best, cls_inds.astype(np.int32), keep
